# revision 7
# baseline (speedup 1.0000x reference)
"""Trainium2 Bass kernel for staircase-sparse varlen GQA attention + paged KV-cache store.

Problem (hardcoded shapes):
  q [8192,16,128] f32, k/v [8192,4,128] f32, k_cache/v_cache [16384,4,128] f32,
  slot_mapping arange(8192) i32, cu_seqlens arange(9)*1024 i32, block_size=128.
Returns (o [8192,2048] f32, k_cache_new, v_cache_new).

Sharding: data-parallel over the B=8 sequences (one per NeuronCore); the
KV-cache scatter + untouched-region copy is sharded over cores as well.

Per-core device kernel:
  - q/k loaded with an f32->f16 casting DMA (SWDGE), transposed on TensorE
    (identity matmul) into [d, token] layout.
  - S^T[kpos, q] = kT_j^T . qT  on TensorE (fp16 in, fp32 PSUM out).
  - p^T = exp(SCALE * S^T) on ScalarE (PSUM f32 -> SBUF f16), batched wide.
  - o[q, 0:128] and softmax denominator o[q, 128] accumulate in one matmul:
    lhsT = p^T, rhs = [v_j | ones]  (ones-column trick), PSUM accumulation
    over the staircase j<=i.
  - normalize with DVE reciprocal + tensor_scalar_mul, DMA out.
  - cache update: DRAM->DRAM DMA copies (touched slots from k/v inputs,
    untouched slots passed through).
"""

import numpy as np

import concourse.bass as bass
import concourse.mybir as mybir
import concourse.tile as tile
from concourse import bacc, bass_utils

# ---- problem constants (hardcoded per harness contract) ----
B, S, H, KV, D = 8, 1024, 16, 4, 128
T = B * S
NUM_SLOTS = 16384
BLOCK = 128
NBLK = S // BLOCK          # 8 staircase blocks per sequence
G = H // KV                # 4 query heads per kv head
SCALE = 0.08838834764831845
N_CORES = 8
UNTOUCHED = NUM_SLOTS - T          # 8192 slots keep their old cache value
UN_PER_CORE = UNTOUCHED // N_CORES  # 1024
VST = 132                  # vb column stride per (kv, j): 128 v cols + 1 ones + pad

F32 = mybir.dt.float32
F16 = mybir.dt.float16

def _emit(nc, tc):
    q_d = nc.dram_tensor("q", [S, H, D], F32, kind="ExternalInput").ap()
    k_d = nc.dram_tensor("k", [S, KV, D], F32, kind="ExternalInput").ap()
    v_d = nc.dram_tensor("v", [S, KV, D], F32, kind="ExternalInput").ap()
    kcu_d = nc.dram_tensor("kc_un", [UN_PER_CORE, KV, D], F32, kind="ExternalInput").ap()
    vcu_d = nc.dram_tensor("vc_un", [UN_PER_CORE, KV, D], F32, kind="ExternalInput").ap()
    o_d = nc.dram_tensor("o", [S, H * D], F32, kind="ExternalOutput").ap()
    kco_d = nc.dram_tensor("kc_out", [S + UN_PER_CORE, KV, D], F32, kind="ExternalOutput").ap()
    vco_d = nc.dram_tensor("vc_out", [S + UN_PER_CORE, KV, D], F32, kind="ExternalOutput").ap()

    # cache copy jobs, chunked so they spread across the kernel
    cache_jobs = []
    n_chunks = 2
    rows = S // n_chunks
    for dst, src, base in ((kco_d, k_d, 0), (kco_d, kcu_d, S),
                           (vco_d, v_d, 0), (vco_d, vcu_d, S)):
        for c in range(n_chunks):
            cache_jobs.append((dst[base + c * rows: base + (c + 1) * rows],
                               src[c * rows: (c + 1) * rows]))

    with (
        tc.tile_pool(name="pers", bufs=1) as pers,
        tc.tile_pool(name="nat", bufs=3) as nat_pool,
        tc.tile_pool(name="pt", bufs=2) as pt_pool,
        tc.tile_pool(name="osb", bufs=4) as osb_pool,
        tc.tile_pool(name="small", bufs=4) as small_pool,
    ):
        # qT is i-block-major: column index = (i*H + h)*128 + p, so that for a
        # fixed q-block i all H heads are contiguous (S^T matmuls span the 4
        # heads of a kv group, N=512).
        qT = pers.tile([128, H * S], F16, tag="qT")
        kT = pers.tile([128, KV * S], F16, tag="kT")
        vb = pers.tile([128, KV * NBLK * VST], F16, tag="vb")

        # ones columns of vb (position 128 in each VST-stride slot)
        vb3 = vb[:].rearrange("p (n x) -> p n x", x=VST)
        nc.vector.memset(vb3[:, :, D:D + 1], 1.0)

        # ---- k: casting load (f32->f16, SWDGE) + xbar DMA transpose ----
        for kv in range(KV):
            knat = nat_pool.tile([128, NBLK * D], F16, tag="nat")
            knat3 = knat[:].rearrange("p (j d) -> p j d", j=NBLK)
            nc.gpsimd.dma_start(knat3, k_d[:, kv, :].rearrange("(j p) d -> p j d", p=128))
            for j in range(NBLK):
                nc.sync.dma_start(kT[:, kv * S + j * BLOCK: kv * S + (j + 1) * BLOCK],
                                  knat[:, j * D:(j + 1) * D], transpose=True)

        # ---- v: load with cast directly into vb slots ----
        for kv in range(KV):
            dst = vb[:, kv * NBLK * VST: (kv + 1) * NBLK * VST]
            dst3 = dst.rearrange("p (j x) -> p j x", j=NBLK)[:, :, 0:D]
            nc.gpsimd.dma_start(dst3, v_d[:, kv, :].rearrange("(j p) d -> p j d", p=128))

        # ---- q: casting load + xbar DMA transpose ----
        for h in range(H):
            qnat = nat_pool.tile([128, NBLK * D], F16, tag="nat")
            qnat3 = qnat[:].rearrange("p (i d) -> p i d", i=NBLK)
            nc.gpsimd.dma_start(qnat3, q_d[:, h, :].rearrange("(i p) d -> p i d", p=128))
            for i in range(NBLK):
                nc.sync.dma_start(qT[:, (i * H + h) * BLOCK: (i * H + h + 1) * BLOCK],
                                  qnat[:, i * D:(i + 1) * D], transpose=True)

        # ---- attention: 4 heads (one kv group) per pass, i-outer, j in pairs ----
        job_idx = 0
        with (
            tc.tile_pool(name="stp", bufs=2, space="PSUM") as st_pool,
            tc.tile_pool(name="oap", bufs=4, space="PSUM") as oacc_pool,
        ):
            ncols = G * BLOCK  # 512 q columns (4 heads) per S^T matmul
            for kv in range(KV):
                h0 = kv * G
                for i in range(NBLK):
                    oas = [oacc_pool.tile([128, D + 1], F32, tag="oacc",
                                          name=f"oacc_kv{kv}_i{i}_h{h4}") for h4 in range(G)]
                    # j-pairs: each st tile holds 2 j-blocks x 512 cols
                    for jc in range(0, i + 1, 2):
                        jn = min(2, i + 1 - jc)
                        st = st_pool.tile([128, 1024], F32, tag="st")
                        pt = pt_pool.tile([128, 1024], F16, tag="pt")
                        for jj in range(jn):
                            j = jc + jj
                            nc.tensor.matmul(
                                st[:, jj * ncols: (jj + 1) * ncols],
                                lhsT=kT[:, kv * S + j * BLOCK: kv * S + (j + 1) * BLOCK],
                                rhs=qT[:, (i * H + h0) * BLOCK: (i * H + h0 + G) * BLOCK],
                                start=True, stop=True,
                            )
                        nc.scalar.activation(pt[:, :jn * ncols], st[:, :jn * ncols],
                                             mybir.ActivationFunctionType.Exp, scale=SCALE)
                        for h4 in range(G):
                            for jj in range(jn):
                                j = jc + jj
                                vslot = (kv * NBLK + j) * VST
                                nc.tensor.matmul(
                                    oas[h4][:],
                                    lhsT=pt[:, jj * ncols + h4 * BLOCK: jj * ncols + (h4 + 1) * BLOCK],
                                    rhs=vb[:, vslot: vslot + D + 1],
                                    start=(j == 0), stop=(j == i),
                                )
                    for h4 in range(G):
                        h = h0 + h4
                        oa = oas[h4]
                        rcp = small_pool.tile([128, 1], F32, tag="rcp")
                        nc.vector.reciprocal(rcp[:], oa[:, D: D + 1])
                        osb = osb_pool.tile([128, D], F32, tag="osb")
                        nc.vector.tensor_scalar_mul(osb[:], oa[:, 0: D], rcp[:])
                        nc.sync.dma_start(
                            o_d[i * BLOCK:(i + 1) * BLOCK, h * D:(h + 1) * D], osb[:])

                    # spread cache copies through the attention loop
                    if job_idx < len(cache_jobs) and i % 2 == 1:
                        dst, src = cache_jobs[job_idx]
                        nc.sync.dma_start(dst, src)
                        job_idx += 1
            # any leftover cache jobs
            while job_idx < len(cache_jobs):
                dst, src = cache_jobs[job_idx]
                nc.sync.dma_start(dst, src)
                job_idx += 1


_PROG = None


def build_program():
    global _PROG
    if _PROG is None:
        nc = bacc.Bacc("TRN2", target_bir_lowering=False, debug=False,
                       num_devices=N_CORES)
        with tile.TileContext(nc) as tc:
            _emit(nc, tc)
        nc.compile()
        _PROG = nc
    return _PROG


def make_in_maps(q, k, v, k_cache, v_cache):
    in_maps = []
    for c in range(N_CORES):
        sl = slice(c * S, (c + 1) * S)
        un = slice(T + c * UN_PER_CORE, T + (c + 1) * UN_PER_CORE)
        in_maps.append({
            "q": np.ascontiguousarray(q[sl]),
            "k": np.ascontiguousarray(k[sl]),
            "v": np.ascontiguousarray(v[sl]),
            "kc_un": np.ascontiguousarray(k_cache[un]),
            "vc_un": np.ascontiguousarray(v_cache[un]),
        })
    return in_maps


def _gather(results, k_cache, v_cache):
    o = np.concatenate([results[c]["o"] for c in range(N_CORES)], axis=0)
    kc = np.empty((NUM_SLOTS, KV, D), np.float32)
    vc = np.empty((NUM_SLOTS, KV, D), np.float32)
    for c in range(N_CORES):
        kc[c * S:(c + 1) * S] = results[c]["kc_out"][:S]
        vc[c * S:(c + 1) * S] = results[c]["vc_out"][:S]
        kc[T + c * UN_PER_CORE: T + (c + 1) * UN_PER_CORE] = results[c]["kc_out"][S:]
        vc[T + c * UN_PER_CORE: T + (c + 1) * UN_PER_CORE] = results[c]["vc_out"][S:]
    return o, kc, vc


def _numpy_fallback(q, k, v, k_cache, v_cache, slot_mapping, cu_seqlens_q):
    """Exact reference semantics in numpy, used only if inputs deviate from
    the hardcoded fast-path layout."""
    kc = k_cache.copy()
    vc = v_cache.copy()
    kc[slot_mapping] = k
    vc[slot_mapping] = v
    b = cu_seqlens_q.shape[0] - 1
    s = q.shape[0] // b
    qb = q.reshape(b, s, H, D)
    kb = np.repeat(k.reshape(b, s, KV, D), G, axis=2)
    vb_ = np.repeat(v.reshape(b, s, KV, D), G, axis=2)
    blk = np.arange(s) // BLOCK
    mask = blk[:, None] >= blk[None, :]
    scores = np.einsum("bqhd,bkhd->bhqk", qb * SCALE, kb)
    scores = np.where(mask[None, None], scores, np.finfo(np.float32).min)
    scores = scores - scores.max(-1, keepdims=True)
    p = np.exp(scores)
    p = p / p.sum(-1, keepdims=True)
    o = np.einsum("bhqk,bkhd->bqhd", p, vb_).astype(np.float32)
    return o.reshape(b * s, H * D), kc, vc


def kernel(q, k, v, k_cache, v_cache, slot_mapping, cu_seqlens_q, cu_seqlens_k,
           block_size):
    q = np.asarray(q, np.float32)
    k = np.asarray(k, np.float32)
    v = np.asarray(v, np.float32)
    k_cache = np.asarray(k_cache, np.float32)
    v_cache = np.asarray(v_cache, np.float32)
    slot_mapping = np.asarray(slot_mapping)
    cu_seqlens_q = np.asarray(cu_seqlens_q)
    cu_seqlens_k = np.asarray(cu_seqlens_k)

    fast = (
        q.shape == (T, H, D) and k.shape == (T, KV, D) and v.shape == (T, KV, D)
        and k_cache.shape == (NUM_SLOTS, KV, D) and v_cache.shape == (NUM_SLOTS, KV, D)
        and int(np.asarray(block_size)) == BLOCK
        and np.array_equal(slot_mapping, np.arange(T, dtype=slot_mapping.dtype))
        and np.array_equal(cu_seqlens_q, np.arange(B + 1, dtype=cu_seqlens_q.dtype) * S)
        and np.array_equal(cu_seqlens_k, np.arange(B + 1, dtype=cu_seqlens_k.dtype) * S)
    )
    if not fast:
        return _numpy_fallback(q, k, v, k_cache, v_cache, slot_mapping, cu_seqlens_q)

    nc = build_program()
    in_maps = make_in_maps(q, k, v, k_cache, v_cache)
    res = bass_utils.run_bass_kernel_spmd(nc, in_maps, core_ids=list(range(N_CORES)))
    return _gather(res.results, k_cache, v_cache)


if __name__ == "__main__":
    rng = np.random.default_rng(0)
    q = rng.standard_normal((T, H, D), dtype=np.float32)
    k = rng.standard_normal((T, KV, D), dtype=np.float32)
    v = rng.standard_normal((T, KV, D), dtype=np.float32)
    kc = np.zeros((NUM_SLOTS, KV, D), np.float32)
    vc = np.zeros((NUM_SLOTS, KV, D), np.float32)
    sm = np.arange(T, dtype=np.int32)
    cu = np.arange(B + 1, dtype=np.int32) * S
    o, kcn, vcn = kernel(q=q, k=k, v=v, k_cache=kc, v_cache=vc, slot_mapping=sm,
                         cu_seqlens_q=cu, cu_seqlens_k=cu, block_size=128)
    oref, kref, vref = _numpy_fallback(q, k, v, kc, vc, sm, cu)
    print("o relerr:", np.abs(o - oref).max() / np.abs(oref).max())
    print("kc equal:", np.array_equal(kcn, kref), "vc equal:", np.array_equal(vcn, vref))


# revision 10
# speedup vs baseline: 1.9212x; 1.9212x over previous
"""Trainium2 Bass kernel for staircase-sparse varlen GQA attention + paged KV-cache store.

Problem (hardcoded shapes):
  q [8192,16,128] f32, k/v [8192,4,128] f32, k_cache/v_cache [16384,4,128] f32,
  slot_mapping arange(8192) i32, cu_seqlens arange(9)*1024 i32, block_size=128.
Returns (o [8192,2048] f32, k_cache_new, v_cache_new).

Sharding: data-parallel over the B=8 sequences (one per NeuronCore); the
KV-cache scatter + untouched-region copy is sharded over cores as well.

Per-core device kernel:
  - q/k loaded with an f32->f16 casting DMA (SWDGE), transposed on TensorE
    (identity matmul) into [d, token] layout.
  - S^T[kpos, q] = kT_j^T . qT  on TensorE (fp16 in, fp32 PSUM out).
  - p^T = exp(SCALE * S^T) on ScalarE (PSUM f32 -> SBUF f16), batched wide.
  - o[q, 0:128] and softmax denominator o[q, 128] accumulate in one matmul:
    lhsT = p^T, rhs = [v_j | ones]  (ones-column trick), PSUM accumulation
    over the staircase j<=i.
  - normalize with DVE reciprocal + tensor_scalar_mul, DMA out.
  - cache update: DRAM->DRAM DMA copies (touched slots from k/v inputs,
    untouched slots passed through).
"""

import numpy as np

import concourse.bass as bass
import concourse.mybir as mybir
import concourse.tile as tile
from concourse import bacc, bass_utils
from concourse.masks import make_identity

# ---- problem constants (hardcoded per harness contract) ----
B, S, H, KV, D = 8, 1024, 16, 4, 128
T = B * S
NUM_SLOTS = 16384
BLOCK = 128
NBLK = S // BLOCK          # 8 staircase blocks per sequence
G = H // KV                # 4 query heads per kv head
SCALE = 0.08838834764831845
N_CORES = 8
UNTOUCHED = NUM_SLOTS - T          # 8192 slots keep their old cache value
UN_PER_CORE = UNTOUCHED // N_CORES  # 1024
VST = 132                  # vb column stride per (kv, j): 128 v cols + 1 ones + pad

F32 = mybir.dt.float32
F16 = mybir.dt.float16

def _emit(nc, tc):
    q_d = nc.dram_tensor("q", [S, H, D], F32, kind="ExternalInput").ap()
    k_d = nc.dram_tensor("k", [S, KV, D], F32, kind="ExternalInput").ap()
    v_d = nc.dram_tensor("v", [S, KV, D], F32, kind="ExternalInput").ap()
    kcu_d = nc.dram_tensor("kc_un", [UN_PER_CORE, KV, D], F32, kind="ExternalInput").ap()
    vcu_d = nc.dram_tensor("vc_un", [UN_PER_CORE, KV, D], F32, kind="ExternalInput").ap()
    o_d = nc.dram_tensor("o", [S, H * D], F32, kind="ExternalOutput").ap()
    kco_d = nc.dram_tensor("kc_out", [S + UN_PER_CORE, KV, D], F32, kind="ExternalOutput").ap()
    vco_d = nc.dram_tensor("vc_out", [S + UN_PER_CORE, KV, D], F32, kind="ExternalOutput").ap()

    # cache copy jobs, chunked so they spread across the kernel
    cache_jobs = []
    n_chunks = 2
    rows = S // n_chunks
    for dst, src, base in ((kco_d, k_d, 0), (kco_d, kcu_d, S),
                           (vco_d, v_d, 0), (vco_d, vcu_d, S)):
        for c in range(n_chunks):
            cache_jobs.append((dst[base + c * rows: base + (c + 1) * rows],
                               src[c * rows: (c + 1) * rows]))

    with (
        tc.tile_pool(name="pers", bufs=1) as pers,
        tc.tile_pool(name="nat", bufs=3) as nat_pool,
        tc.tile_pool(name="pt", bufs=2) as pt_pool,
        tc.tile_pool(name="osb", bufs=4) as osb_pool,
        tc.tile_pool(name="small", bufs=4) as small_pool,
    ):
        # qT is i-block-major: column index = (i*H + h)*128 + p, so that for a
        # fixed q-block i all H heads are contiguous (S^T matmuls span the 4
        # heads of a kv group, N=512).
        qT = pers.tile([128, H * S], F16, tag="qT")
        kT = pers.tile([128, KV * S], F16, tag="kT")
        vb = pers.tile([128, KV * NBLK * VST], F16, tag="vb")

        # ones columns of vb (position 128 in each VST-stride slot)
        vb3 = vb[:].rearrange("p (n x) -> p n x", x=VST)
        nc.vector.memset(vb3[:, :, D:D + 1], 1.0)

        ident = pers.tile([128, 128], F16, tag="ident")
        make_identity(nc, ident[:])

        with tc.tile_pool(name="tp", bufs=4, space="PSUM") as tp_pool:
            # ---- k: casting load (f32->f16, SWDGE) + TensorE transpose ----
            for kv in range(KV):
                knat = nat_pool.tile([128, NBLK * D], F16, tag="nat")
                knat3 = knat[:].rearrange("p (j d) -> p j d", j=NBLK)
                nc.gpsimd.dma_start(knat3, k_d[:, kv, :].rearrange("(j p) d -> p j d", p=128))
                for j in range(NBLK):
                    tp = tp_pool.tile([128, 128], F16, tag="tp")
                    nc.tensor.transpose(tp[:], knat[:, j * D:(j + 1) * D], ident[:])
                    nc.vector.tensor_copy(kT[:, kv * S + j * BLOCK: kv * S + (j + 1) * BLOCK], tp[:])

            # ---- v: load with cast directly into vb slots ----
            for kv in range(KV):
                dst = vb[:, kv * NBLK * VST: (kv + 1) * NBLK * VST]
                dst3 = dst.rearrange("p (j x) -> p j x", j=NBLK)[:, :, 0:D]
                nc.gpsimd.dma_start(dst3, v_d[:, kv, :].rearrange("(j p) d -> p j d", p=128))

            # ---- q: casting load + TensorE transpose ----
            for h in range(H):
                qnat = nat_pool.tile([128, NBLK * D], F16, tag="nat")
                qnat3 = qnat[:].rearrange("p (i d) -> p i d", i=NBLK)
                nc.gpsimd.dma_start(qnat3, q_d[:, h, :].rearrange("(i p) d -> p i d", p=128))
                for i in range(NBLK):
                    tp = tp_pool.tile([128, 128], F16, tag="tp")
                    nc.tensor.transpose(tp[:], qnat[:, i * D:(i + 1) * D], ident[:])
                    nc.vector.tensor_copy(
                        qT[:, (i * H + h) * BLOCK: (i * H + h + 1) * BLOCK], tp[:])

        # ---- attention: 4 heads (one kv group) per pass, i-outer, j in pairs.
        # Emission is software-pipelined one chunk deep: chunk n's S^T matmuls
        # are emitted BEFORE chunk n-1's exp/PV, so TensorE always has
        # independent work while ScalarE runs exp.
        job_idx = 0
        with (
            tc.tile_pool(name="stp", bufs=2, space="PSUM") as st_pool,
            tc.tile_pool(name="oap", bufs=4, space="PSUM") as oacc_pool,
        ):
            ncols = G * BLOCK  # 512 q columns (4 heads) per S^T matmul

            # chunk list: (kv, i, jc, jn, first, last)
            chunks = []
            for kv in range(KV):
                for i in range(NBLK):
                    jcs = list(range(0, i + 1, 2))
                    for jc in jcs:
                        chunks.append((kv, i, jc, min(2, i + 1 - jc),
                                       jc == 0, jc == jcs[-1]))

            oas_map = {}
            pending = None  # (chunk, st, pt)

            def emit_stage2(chunk, st, pt):
                """exp + PV (+ normalize/store on last chunk of an i-row)."""
                nonlocal job_idx
                kv, i, jc, jn, first, last = chunk
                h0 = kv * G
                nc.scalar.activation(pt[:, :jn * ncols], st[:, :jn * ncols],
                                     mybir.ActivationFunctionType.Exp, scale=SCALE)
                oas = oas_map[(kv, i)]
                for h4 in range(G):
                    for jj in range(jn):
                        j = jc + jj
                        vslot = (kv * NBLK + j) * VST
                        nc.tensor.matmul(
                            oas[h4][:],
                            lhsT=pt[:, jj * ncols + h4 * BLOCK: jj * ncols + (h4 + 1) * BLOCK],
                            rhs=vb[:, vslot: vslot + D + 1],
                            start=(j == 0), stop=(j == i),
                        )
                if last:
                    for h4 in range(G):
                        h = h0 + h4
                        oa = oas[h4]
                        rcp = small_pool.tile([128, 1], F32, tag="rcp", name=f"rcp_{kv}_{i}_{h4}")
                        nc.vector.reciprocal(rcp[:], oa[:, D: D + 1])
                        osb = osb_pool.tile([128, D], F32, tag="osb", name=f"osb_{kv}_{i}_{h4}")
                        nc.vector.tensor_scalar_mul(osb[:], oa[:, 0: D], rcp[:])
                        nc.sync.dma_start(
                            o_d[i * BLOCK:(i + 1) * BLOCK, h * D:(h + 1) * D], osb[:])
                    del oas_map[(kv, i)]
                    if job_idx < len(cache_jobs) and i % 2 == 1:
                        dst, src = cache_jobs[job_idx]
                        nc.sync.dma_start(dst, src)
                        job_idx += 1

            for chunk in chunks:
                kv, i, jc, jn, first, last = chunk
                h0 = kv * G
                if first:
                    oas_map[(kv, i)] = [
                        oacc_pool.tile([128, D + 1], F32, tag="oacc",
                                       name=f"oacc_kv{kv}_i{i}_h{h4}") for h4 in range(G)]
                st = st_pool.tile([128, 1024], F32, tag="st", name=f"st_{kv}_{i}_{jc}")
                pt = pt_pool.tile([128, 1024], F16, tag="pt", name=f"pt_{kv}_{i}_{jc}")
                for jj in range(jn):
                    j = jc + jj
                    nc.tensor.matmul(
                        st[:, jj * ncols: (jj + 1) * ncols],
                        lhsT=kT[:, kv * S + j * BLOCK: kv * S + (j + 1) * BLOCK],
                        rhs=qT[:, (i * H + h0) * BLOCK: (i * H + h0 + G) * BLOCK],
                        start=True, stop=True,
                    )
                if pending is not None:
                    emit_stage2(*pending)
                pending = (chunk, st, pt)
            if pending is not None:
                emit_stage2(*pending)

            # any leftover cache jobs
            while job_idx < len(cache_jobs):
                dst, src = cache_jobs[job_idx]
                nc.sync.dma_start(dst, src)
                job_idx += 1


_PROG = None


def build_program():
    global _PROG
    if _PROG is None:
        nc = bacc.Bacc("TRN2", target_bir_lowering=False, debug=False,
                       num_devices=N_CORES)
        with tile.TileContext(nc) as tc:
            _emit(nc, tc)
        nc.compile()
        _PROG = nc
    return _PROG


def make_in_maps(q, k, v, k_cache, v_cache):
    in_maps = []
    for c in range(N_CORES):
        sl = slice(c * S, (c + 1) * S)
        un = slice(T + c * UN_PER_CORE, T + (c + 1) * UN_PER_CORE)
        in_maps.append({
            "q": np.ascontiguousarray(q[sl]),
            "k": np.ascontiguousarray(k[sl]),
            "v": np.ascontiguousarray(v[sl]),
            "kc_un": np.ascontiguousarray(k_cache[un]),
            "vc_un": np.ascontiguousarray(v_cache[un]),
        })
    return in_maps


def _gather(results, k_cache, v_cache):
    o = np.concatenate([results[c]["o"] for c in range(N_CORES)], axis=0)
    kc = np.empty((NUM_SLOTS, KV, D), np.float32)
    vc = np.empty((NUM_SLOTS, KV, D), np.float32)
    for c in range(N_CORES):
        kc[c * S:(c + 1) * S] = results[c]["kc_out"][:S]
        vc[c * S:(c + 1) * S] = results[c]["vc_out"][:S]
        kc[T + c * UN_PER_CORE: T + (c + 1) * UN_PER_CORE] = results[c]["kc_out"][S:]
        vc[T + c * UN_PER_CORE: T + (c + 1) * UN_PER_CORE] = results[c]["vc_out"][S:]
    return o, kc, vc


def _numpy_fallback(q, k, v, k_cache, v_cache, slot_mapping, cu_seqlens_q):
    """Exact reference semantics in numpy, used only if inputs deviate from
    the hardcoded fast-path layout."""
    kc = k_cache.copy()
    vc = v_cache.copy()
    kc[slot_mapping] = k
    vc[slot_mapping] = v
    b = cu_seqlens_q.shape[0] - 1
    s = q.shape[0] // b
    qb = q.reshape(b, s, H, D)
    kb = np.repeat(k.reshape(b, s, KV, D), G, axis=2)
    vb_ = np.repeat(v.reshape(b, s, KV, D), G, axis=2)
    blk = np.arange(s) // BLOCK
    mask = blk[:, None] >= blk[None, :]
    scores = np.einsum("bqhd,bkhd->bhqk", qb * SCALE, kb)
    scores = np.where(mask[None, None], scores, np.finfo(np.float32).min)
    scores = scores - scores.max(-1, keepdims=True)
    p = np.exp(scores)
    p = p / p.sum(-1, keepdims=True)
    o = np.einsum("bhqk,bkhd->bqhd", p, vb_).astype(np.float32)
    return o.reshape(b * s, H * D), kc, vc


def kernel(q, k, v, k_cache, v_cache, slot_mapping, cu_seqlens_q, cu_seqlens_k,
           block_size):
    q = np.asarray(q, np.float32)
    k = np.asarray(k, np.float32)
    v = np.asarray(v, np.float32)
    k_cache = np.asarray(k_cache, np.float32)
    v_cache = np.asarray(v_cache, np.float32)
    slot_mapping = np.asarray(slot_mapping)
    cu_seqlens_q = np.asarray(cu_seqlens_q)
    cu_seqlens_k = np.asarray(cu_seqlens_k)

    fast = (
        q.shape == (T, H, D) and k.shape == (T, KV, D) and v.shape == (T, KV, D)
        and k_cache.shape == (NUM_SLOTS, KV, D) and v_cache.shape == (NUM_SLOTS, KV, D)
        and int(np.asarray(block_size)) == BLOCK
        and np.array_equal(slot_mapping, np.arange(T, dtype=slot_mapping.dtype))
        and np.array_equal(cu_seqlens_q, np.arange(B + 1, dtype=cu_seqlens_q.dtype) * S)
        and np.array_equal(cu_seqlens_k, np.arange(B + 1, dtype=cu_seqlens_k.dtype) * S)
    )
    if not fast:
        return _numpy_fallback(q, k, v, k_cache, v_cache, slot_mapping, cu_seqlens_q)

    nc = build_program()
    in_maps = make_in_maps(q, k, v, k_cache, v_cache)
    res = bass_utils.run_bass_kernel_spmd(nc, in_maps, core_ids=list(range(N_CORES)))
    return _gather(res.results, k_cache, v_cache)


if __name__ == "__main__":
    rng = np.random.default_rng(0)
    q = rng.standard_normal((T, H, D), dtype=np.float32)
    k = rng.standard_normal((T, KV, D), dtype=np.float32)
    v = rng.standard_normal((T, KV, D), dtype=np.float32)
    kc = np.zeros((NUM_SLOTS, KV, D), np.float32)
    vc = np.zeros((NUM_SLOTS, KV, D), np.float32)
    sm = np.arange(T, dtype=np.int32)
    cu = np.arange(B + 1, dtype=np.int32) * S
    o, kcn, vcn = kernel(q=q, k=k, v=v, k_cache=kc, v_cache=vc, slot_mapping=sm,
                         cu_seqlens_q=cu, cu_seqlens_k=cu, block_size=128)
    oref, kref, vref = _numpy_fallback(q, k, v, kc, vc, sm, cu)
    print("o relerr:", np.abs(o - oref).max() / np.abs(oref).max())
    print("kc equal:", np.array_equal(kcn, kref), "vc equal:", np.array_equal(vcn, vref))


# revision 12
# speedup vs baseline: 2.2306x; 1.1610x over previous
"""Trainium2 Bass kernel for staircase-sparse varlen GQA attention + paged KV-cache store.

Problem (hardcoded shapes):
  q [8192,16,128] f32, k/v [8192,4,128] f32, k_cache/v_cache [16384,4,128] f32,
  slot_mapping arange(8192) i32, cu_seqlens arange(9)*1024 i32, block_size=128.
Returns (o [8192,2048] f32, k_cache_new, v_cache_new).

Sharding: data-parallel over the B=8 sequences (one per NeuronCore); the
KV-cache scatter + untouched-region copy is sharded over cores as well.

Per-core device kernel:
  - q/k loaded with an f32->f16 casting DMA (SWDGE), transposed on TensorE
    (identity matmul) into [d, token] layout.
  - S^T[kpos, q] = kT_j^T . qT  on TensorE (fp16 in, fp32 PSUM out).
  - p^T = exp(SCALE * S^T) on ScalarE (PSUM f32 -> SBUF f16), batched wide.
  - o[q, 0:128] and softmax denominator o[q, 128] accumulate in one matmul:
    lhsT = p^T, rhs = [v_j | ones]  (ones-column trick), PSUM accumulation
    over the staircase j<=i.
  - normalize with DVE reciprocal + tensor_scalar_mul, DMA out.
  - cache update: DRAM->DRAM DMA copies (touched slots from k/v inputs,
    untouched slots passed through).
"""

import numpy as np

import concourse.bass as bass
import concourse.mybir as mybir
import concourse.tile as tile
from concourse import bacc, bass_utils
from concourse.masks import make_identity

# ---- problem constants (hardcoded per harness contract) ----
B, S, H, KV, D = 8, 1024, 16, 4, 128
T = B * S
NUM_SLOTS = 16384
BLOCK = 128
NBLK = S // BLOCK          # 8 staircase blocks per sequence
G = H // KV                # 4 query heads per kv head
SCALE = 0.08838834764831845
N_CORES = 8
UNTOUCHED = NUM_SLOTS - T          # 8192 slots keep their old cache value
UN_PER_CORE = UNTOUCHED // N_CORES  # 1024
VST = 132                  # vb column stride per (kv, j): 128 v cols + 1 ones + pad

F32 = mybir.dt.float32
F16 = mybir.dt.float16

def _emit(nc, tc):
    q_d = nc.dram_tensor("q", [S, H, D], F32, kind="ExternalInput").ap()
    k_d = nc.dram_tensor("k", [S, KV, D], F32, kind="ExternalInput").ap()
    v_d = nc.dram_tensor("v", [S, KV, D], F32, kind="ExternalInput").ap()
    kcu_d = nc.dram_tensor("kc_un", [UN_PER_CORE, KV, D], F32, kind="ExternalInput").ap()
    vcu_d = nc.dram_tensor("vc_un", [UN_PER_CORE, KV, D], F32, kind="ExternalInput").ap()
    o_d = nc.dram_tensor("o", [S, H * D], F32, kind="ExternalOutput").ap()
    kco_d = nc.dram_tensor("kc_out", [S + UN_PER_CORE, KV, D], F32, kind="ExternalOutput").ap()
    vco_d = nc.dram_tensor("vc_out", [S + UN_PER_CORE, KV, D], F32, kind="ExternalOutput").ap()

    # cache copy jobs, chunked so they spread across the kernel
    cache_jobs = []
    n_chunks = 2
    rows = S // n_chunks
    for dst, src, base in ((kco_d, k_d, 0), (kco_d, kcu_d, S),
                           (vco_d, v_d, 0), (vco_d, vcu_d, S)):
        for c in range(n_chunks):
            cache_jobs.append((dst[base + c * rows: base + (c + 1) * rows],
                               src[c * rows: (c + 1) * rows]))

    with (
        tc.tile_pool(name="pers", bufs=1) as pers,
        tc.tile_pool(name="nat", bufs=3) as nat_pool,
        tc.tile_pool(name="pt", bufs=2) as pt_pool,
        tc.tile_pool(name="osb", bufs=4) as osb_pool,
        tc.tile_pool(name="small", bufs=4) as small_pool,
    ):
        # qT is i-block-major: column index = (i*H + h)*128 + p, so that for a
        # fixed q-block i all H heads are contiguous (S^T matmuls span the 4
        # heads of a kv group, N=512).
        qT = pers.tile([128, H * S], F16, tag="qT")
        kT = pers.tile([128, KV * S], F16, tag="kT")
        vb = pers.tile([128, KV * NBLK * VST], F16, tag="vb")

        # ones columns of vb (position 128 in each VST-stride slot)
        vb3 = vb[:].rearrange("p (n x) -> p n x", x=VST)
        nc.vector.memset(vb3[:, :, D:D + 1], 1.0)

        ident = pers.tile([128, 128], F16, tag="ident")
        make_identity(nc, ident[:])

        with (
            tc.tile_pool(name="natf", bufs=3) as natf_pool,
            tc.tile_pool(name="tp", bufs=3, space="PSUM") as tp_pool,
        ):
            def load_cast(src_ap, name):
                """HWDGE f32 load of [S, D] (one head) -> SBUF f16 [128, 8*128]."""
                f32t = natf_pool.tile([128, NBLK * D], F32, tag="natf", name=f"f32_{name}")
                nc.sync.dma_start(f32t[:].rearrange("p (j d) -> p j d", j=NBLK),
                                  src_ap.rearrange("(j p) d -> p j d", p=128))
                f16t = nat_pool.tile([128, NBLK * D], F16, tag="nat", name=f"f16_{name}")
                nc.vector.tensor_copy(f16t[:], f32t[:])
                return f16t

            for kv in range(KV):
                # k: load + cast + 8 transposes into one psum bank + 1 evac
                knat = load_cast(k_d[:, kv, :], f"k{kv}")
                tpk = tp_pool.tile([128, NBLK * BLOCK], F16, tag="tp", name=f"tpk{kv}")
                for j in range(NBLK):
                    nc.tensor.transpose(tpk[:, j * BLOCK:(j + 1) * BLOCK],
                                        knat[:, j * D:(j + 1) * D], ident[:])
                nc.vector.tensor_copy(kT[:, kv * S:(kv + 1) * S], tpk[:])

                # v: load + cast into vb strided slots
                vf32 = natf_pool.tile([128, NBLK * D], F32, tag="natf", name=f"vf32_{kv}")
                nc.sync.dma_start(vf32[:].rearrange("p (j d) -> p j d", j=NBLK),
                                  v_d[:, kv, :].rearrange("(j p) d -> p j d", p=128))
                dst = vb[:, kv * NBLK * VST: (kv + 1) * NBLK * VST]
                dst3 = dst.rearrange("p (j x) -> p j x", j=NBLK)[:, :, 0:D]
                nc.vector.tensor_copy(dst3, vf32[:].rearrange("p (j d) -> p j d", j=NBLK))

                # q heads of this kv group
                for h4 in range(G):
                    h = kv * G + h4
                    qnat = load_cast(q_d[:, h, :], f"q{h}")
                    tpq = tp_pool.tile([128, NBLK * BLOCK], F16, tag="tp", name=f"tpq{h}")
                    for i in range(NBLK):
                        nc.tensor.transpose(tpq[:, i * BLOCK:(i + 1) * BLOCK],
                                            qnat[:, i * D:(i + 1) * D], ident[:])
                    # strided evac: column block i goes to (i*H + h)*128
                    qT3 = qT[:].rearrange("p (i hh b) -> p i (hh b)", i=NBLK, hh=H)
                    nc.vector.tensor_copy(qT3[:, :, h * BLOCK:(h + 1) * BLOCK],
                                          tpq[:].rearrange("p (i b) -> p i b", i=NBLK))

        # ---- attention: 4 heads (one kv group) per pass, i-outer, j in pairs.
        # Emission is software-pipelined one chunk deep: chunk n's S^T matmuls
        # are emitted BEFORE chunk n-1's exp/PV, so TensorE always has
        # independent work while ScalarE runs exp.
        job_idx = 0
        with (
            tc.tile_pool(name="stp", bufs=2, space="PSUM") as st_pool,
            tc.tile_pool(name="oap", bufs=4, space="PSUM") as oacc_pool,
        ):
            ncols = G * BLOCK  # 512 q columns (4 heads) per S^T matmul

            # chunk list: (kv, i, jc, jn, first, last)
            chunks = []
            for kv in range(KV):
                for i in range(NBLK):
                    jcs = list(range(0, i + 1, 2))
                    for jc in jcs:
                        chunks.append((kv, i, jc, min(2, i + 1 - jc),
                                       jc == 0, jc == jcs[-1]))

            oas_map = {}
            pending = None  # (chunk, st, pt)

            def emit_stage2(chunk, st, pt):
                """exp + PV (+ normalize/store on last chunk of an i-row)."""
                nonlocal job_idx
                kv, i, jc, jn, first, last = chunk
                h0 = kv * G
                nc.scalar.activation(pt[:, :jn * ncols], st[:, :jn * ncols],
                                     mybir.ActivationFunctionType.Exp, scale=SCALE)
                oas = oas_map[(kv, i)]
                for h4 in range(G):
                    for jj in range(jn):
                        j = jc + jj
                        vslot = (kv * NBLK + j) * VST
                        nc.tensor.matmul(
                            oas[h4][:],
                            lhsT=pt[:, jj * ncols + h4 * BLOCK: jj * ncols + (h4 + 1) * BLOCK],
                            rhs=vb[:, vslot: vslot + D + 1],
                            start=(j == 0), stop=(j == i),
                        )
                if last:
                    osb = osb_pool.tile([128, G * D], F32, tag="osb", name=f"osb_{kv}_{i}")
                    for h4 in range(G):
                        oa = oas[h4]
                        rcp = small_pool.tile([128, 1], F32, tag="rcp", name=f"rcp_{kv}_{i}_{h4}")
                        nc.vector.reciprocal(rcp[:], oa[:, D: D + 1])
                        nc.vector.tensor_scalar_mul(osb[:, h4 * D:(h4 + 1) * D], oa[:, 0: D], rcp[:])
                    nc.sync.dma_start(
                        o_d[i * BLOCK:(i + 1) * BLOCK, h0 * D:(h0 + G) * D], osb[:])
                    del oas_map[(kv, i)]
                    if job_idx < len(cache_jobs) and i % 2 == 1:
                        dst, src = cache_jobs[job_idx]
                        nc.sync.dma_start(dst, src)
                        job_idx += 1

            for chunk in chunks:
                kv, i, jc, jn, first, last = chunk
                h0 = kv * G
                if first:
                    oas_map[(kv, i)] = [
                        oacc_pool.tile([128, D + 1], F32, tag="oacc",
                                       name=f"oacc_kv{kv}_i{i}_h{h4}") for h4 in range(G)]
                st = st_pool.tile([128, 1024], F32, tag="st", name=f"st_{kv}_{i}_{jc}")
                pt = pt_pool.tile([128, 1024], F16, tag="pt", name=f"pt_{kv}_{i}_{jc}")
                for jj in range(jn):
                    j = jc + jj
                    nc.tensor.matmul(
                        st[:, jj * ncols: (jj + 1) * ncols],
                        lhsT=kT[:, kv * S + j * BLOCK: kv * S + (j + 1) * BLOCK],
                        rhs=qT[:, (i * H + h0) * BLOCK: (i * H + h0 + G) * BLOCK],
                        start=True, stop=True,
                    )
                if pending is not None:
                    emit_stage2(*pending)
                pending = (chunk, st, pt)
            if pending is not None:
                emit_stage2(*pending)

            # any leftover cache jobs
            while job_idx < len(cache_jobs):
                dst, src = cache_jobs[job_idx]
                nc.sync.dma_start(dst, src)
                job_idx += 1


_PROG = None


def build_program():
    global _PROG
    if _PROG is None:
        nc = bacc.Bacc("TRN2", target_bir_lowering=False, debug=False,
                       num_devices=N_CORES)
        with tile.TileContext(nc) as tc:
            _emit(nc, tc)
        nc.compile()
        _PROG = nc
    return _PROG


def make_in_maps(q, k, v, k_cache, v_cache):
    in_maps = []
    for c in range(N_CORES):
        sl = slice(c * S, (c + 1) * S)
        un = slice(T + c * UN_PER_CORE, T + (c + 1) * UN_PER_CORE)
        in_maps.append({
            "q": np.ascontiguousarray(q[sl]),
            "k": np.ascontiguousarray(k[sl]),
            "v": np.ascontiguousarray(v[sl]),
            "kc_un": np.ascontiguousarray(k_cache[un]),
            "vc_un": np.ascontiguousarray(v_cache[un]),
        })
    return in_maps


def _gather(results, k_cache, v_cache):
    o = np.concatenate([results[c]["o"] for c in range(N_CORES)], axis=0)
    kc = np.empty((NUM_SLOTS, KV, D), np.float32)
    vc = np.empty((NUM_SLOTS, KV, D), np.float32)
    for c in range(N_CORES):
        kc[c * S:(c + 1) * S] = results[c]["kc_out"][:S]
        vc[c * S:(c + 1) * S] = results[c]["vc_out"][:S]
        kc[T + c * UN_PER_CORE: T + (c + 1) * UN_PER_CORE] = results[c]["kc_out"][S:]
        vc[T + c * UN_PER_CORE: T + (c + 1) * UN_PER_CORE] = results[c]["vc_out"][S:]
    return o, kc, vc


def _numpy_fallback(q, k, v, k_cache, v_cache, slot_mapping, cu_seqlens_q):
    """Exact reference semantics in numpy, used only if inputs deviate from
    the hardcoded fast-path layout."""
    kc = k_cache.copy()
    vc = v_cache.copy()
    kc[slot_mapping] = k
    vc[slot_mapping] = v
    b = cu_seqlens_q.shape[0] - 1
    s = q.shape[0] // b
    qb = q.reshape(b, s, H, D)
    kb = np.repeat(k.reshape(b, s, KV, D), G, axis=2)
    vb_ = np.repeat(v.reshape(b, s, KV, D), G, axis=2)
    blk = np.arange(s) // BLOCK
    mask = blk[:, None] >= blk[None, :]
    scores = np.einsum("bqhd,bkhd->bhqk", qb * SCALE, kb)
    scores = np.where(mask[None, None], scores, np.finfo(np.float32).min)
    scores = scores - scores.max(-1, keepdims=True)
    p = np.exp(scores)
    p = p / p.sum(-1, keepdims=True)
    o = np.einsum("bhqk,bkhd->bqhd", p, vb_).astype(np.float32)
    return o.reshape(b * s, H * D), kc, vc


def kernel(q, k, v, k_cache, v_cache, slot_mapping, cu_seqlens_q, cu_seqlens_k,
           block_size):
    q = np.asarray(q, np.float32)
    k = np.asarray(k, np.float32)
    v = np.asarray(v, np.float32)
    k_cache = np.asarray(k_cache, np.float32)
    v_cache = np.asarray(v_cache, np.float32)
    slot_mapping = np.asarray(slot_mapping)
    cu_seqlens_q = np.asarray(cu_seqlens_q)
    cu_seqlens_k = np.asarray(cu_seqlens_k)

    fast = (
        q.shape == (T, H, D) and k.shape == (T, KV, D) and v.shape == (T, KV, D)
        and k_cache.shape == (NUM_SLOTS, KV, D) and v_cache.shape == (NUM_SLOTS, KV, D)
        and int(np.asarray(block_size)) == BLOCK
        and np.array_equal(slot_mapping, np.arange(T, dtype=slot_mapping.dtype))
        and np.array_equal(cu_seqlens_q, np.arange(B + 1, dtype=cu_seqlens_q.dtype) * S)
        and np.array_equal(cu_seqlens_k, np.arange(B + 1, dtype=cu_seqlens_k.dtype) * S)
    )
    if not fast:
        return _numpy_fallback(q, k, v, k_cache, v_cache, slot_mapping, cu_seqlens_q)

    nc = build_program()
    in_maps = make_in_maps(q, k, v, k_cache, v_cache)
    res = bass_utils.run_bass_kernel_spmd(nc, in_maps, core_ids=list(range(N_CORES)))
    return _gather(res.results, k_cache, v_cache)


if __name__ == "__main__":
    rng = np.random.default_rng(0)
    q = rng.standard_normal((T, H, D), dtype=np.float32)
    k = rng.standard_normal((T, KV, D), dtype=np.float32)
    v = rng.standard_normal((T, KV, D), dtype=np.float32)
    kc = np.zeros((NUM_SLOTS, KV, D), np.float32)
    vc = np.zeros((NUM_SLOTS, KV, D), np.float32)
    sm = np.arange(T, dtype=np.int32)
    cu = np.arange(B + 1, dtype=np.int32) * S
    o, kcn, vcn = kernel(q=q, k=k, v=v, k_cache=kc, v_cache=vc, slot_mapping=sm,
                         cu_seqlens_q=cu, cu_seqlens_k=cu, block_size=128)
    oref, kref, vref = _numpy_fallback(q, k, v, kc, vc, sm, cu)
    print("o relerr:", np.abs(o - oref).max() / np.abs(oref).max())
    print("kc equal:", np.array_equal(kcn, kref), "vc equal:", np.array_equal(vcn, vref))


# revision 16
# speedup vs baseline: 2.5204x; 1.1299x over previous
"""Trainium2 Bass kernel for staircase-sparse varlen GQA attention + paged KV-cache store.

Problem (hardcoded shapes):
  q [8192,16,128] f32, k/v [8192,4,128] f32, k_cache/v_cache [16384,4,128] f32,
  slot_mapping arange(8192) i32, cu_seqlens arange(9)*1024 i32, block_size=128.
Returns (o [8192,2048] f32, k_cache_new, v_cache_new).

Sharding: data-parallel over the B=8 sequences (one per NeuronCore); the
KV-cache scatter + untouched-region copy is sharded over cores as well.

Per-core device kernel:
  - q/k loaded with an f32->f16 casting DMA (SWDGE), transposed on TensorE
    (identity matmul) into [d, token] layout.
  - S^T[kpos, q] = kT_j^T . qT  on TensorE (fp16 in, fp32 PSUM out).
  - p^T = exp(SCALE * S^T) on ScalarE (PSUM f32 -> SBUF f16), batched wide.
  - o[q, 0:128] and softmax denominator o[q, 128] accumulate in one matmul:
    lhsT = p^T, rhs = [v_j | ones]  (ones-column trick), PSUM accumulation
    over the staircase j<=i.
  - normalize with DVE reciprocal + tensor_scalar_mul, DMA out.
  - cache update: DRAM->DRAM DMA copies (touched slots from k/v inputs,
    untouched slots passed through).
"""

import numpy as np

import concourse.bass as bass
import concourse.mybir as mybir
import concourse.tile as tile
from concourse import bacc, bass_utils
from concourse.masks import make_identity

# ---- problem constants (hardcoded per harness contract) ----
B, S, H, KV, D = 8, 1024, 16, 4, 128
T = B * S
NUM_SLOTS = 16384
BLOCK = 128
NBLK = S // BLOCK          # 8 staircase blocks per sequence
G = H // KV                # 4 query heads per kv head
SCALE = 0.08838834764831845
N_CORES = 8
UNTOUCHED = NUM_SLOTS - T          # 8192 slots keep their old cache value
UN_PER_CORE = UNTOUCHED // N_CORES  # 1024
VST = 132                  # vb column stride per (kv, j): 128 v cols + 1 ones + pad

F32 = mybir.dt.float32
F16 = mybir.dt.float16

def _emit(nc, tc):
    q_d = nc.dram_tensor("q", [S, H, D], F32, kind="ExternalInput").ap()
    k_d = nc.dram_tensor("k", [S, KV, D], F32, kind="ExternalInput").ap()
    v_d = nc.dram_tensor("v", [S, KV, D], F32, kind="ExternalInput").ap()
    kcu_d = nc.dram_tensor("kc_un", [UN_PER_CORE, KV, D], F32, kind="ExternalInput").ap()
    vcu_d = nc.dram_tensor("vc_un", [UN_PER_CORE, KV, D], F32, kind="ExternalInput").ap()
    o_d = nc.dram_tensor("o", [S, H * D], F32, kind="ExternalOutput").ap()
    kco_d = nc.dram_tensor("kc_out", [S + UN_PER_CORE, KV, D], F32, kind="ExternalOutput").ap()
    vco_d = nc.dram_tensor("vc_out", [S + UN_PER_CORE, KV, D], F32, kind="ExternalOutput").ap()

    # cache copy jobs, chunked so they spread across the kernel
    cache_jobs = []
    n_chunks = 2
    rows = S // n_chunks
    for dst, src, base in ((kco_d, k_d, 0), (kco_d, kcu_d, S),
                           (vco_d, v_d, 0), (vco_d, vcu_d, S)):
        for c in range(n_chunks):
            cache_jobs.append((dst[base + c * rows: base + (c + 1) * rows],
                               src[c * rows: (c + 1) * rows]))

    with (
        tc.tile_pool(name="pers", bufs=1) as pers,
        tc.tile_pool(name="pt", bufs=2) as pt_pool,
        tc.tile_pool(name="osb", bufs=4) as osb_pool,
        tc.tile_pool(name="small", bufs=4) as small_pool,
    ):
        # qT is i-block-major: column index = (i*H + h)*128 + p, so that for a
        # fixed q-block i all H heads are contiguous (S^T matmuls span the 4
        # heads of a kv group, N=512).
        qT = pers.tile([128, H * S], F16, tag="qT")
        kT = pers.tile([128, KV * S], F16, tag="kT")
        vb = pers.tile([128, KV * NBLK * VST], F16, tag="vb")

        # ones columns of vb (position 128 in each VST-stride slot)
        vb3 = vb[:].rearrange("p (n x) -> p n x", x=VST)
        nc.vector.memset(vb3[:, :, D:D + 1], 1.0)

        ident = pers.tile([128, 128], F16, tag="ident")
        make_identity(nc, ident[:])

        # Block-major loads: q rows are [H*D]=8KB contiguous, k/v rows 2KB
        # contiguous -> full-bandwidth DMA descriptors. Block i's transposes
        # produce qT columns for ALL heads at once, so attention rows in
        # i-ascending order can start after block 0.
        with (
            tc.tile_pool(name="natq", bufs=2) as natq_pool,
            tc.tile_pool(name="natkv", bufs=2) as natkv_pool,
            tc.tile_pool(name="tpq", bufs=2, space="PSUM") as tpq_pool,
            tc.tile_pool(name="tpk", bufs=2, space="PSUM") as tpk_pool,
        ):
            for blk in range(NBLK):
                rows = slice(blk * BLOCK, (blk + 1) * BLOCK)
                # ---- q block ----
                qf32 = natq_pool.tile([128, H * D], F32, tag="qf32", name=f"qf32_{blk}")
                nc.sync.dma_start(qf32[:], q_d[rows].rearrange("p h d -> p (h d)"))
                qf16 = natq_pool.tile([128, H * D], F16, tag="qf16", name=f"qf16_{blk}")
                nc.vector.tensor_copy(qf16[:], qf32[:])
                tpq = tpq_pool.tile([128, H * BLOCK], F16, tag="tpq", name=f"tpq_{blk}")
                for h in range(H):
                    nc.tensor.transpose(tpq[:, h * BLOCK:(h + 1) * BLOCK],
                                        qf16[:, h * D:(h + 1) * D], ident[:])
                # contiguous destination: all heads of q-block `blk`
                nc.scalar.copy(qT[:, blk * H * BLOCK:(blk + 1) * H * BLOCK], tpq[:])

                # ---- k block ----
                kf32 = natkv_pool.tile([128, KV * D], F32, tag="kf32", name=f"kf32_{blk}")
                nc.sync.dma_start(kf32[:], k_d[rows].rearrange("p c d -> p (c d)"))
                kf16 = natkv_pool.tile([128, KV * D], F16, tag="kf16", name=f"kf16_{blk}")
                nc.vector.tensor_copy(kf16[:], kf32[:])
                tpk = tpk_pool.tile([128, KV * BLOCK], F16, tag="tpk", name=f"tpk_{blk}")
                for kv in range(KV):
                    nc.tensor.transpose(tpk[:, kv * BLOCK:(kv + 1) * BLOCK],
                                        kf16[:, kv * D:(kv + 1) * D], ident[:])
                kT4 = kT[:].rearrange("p (c s) -> p c s", c=KV)
                nc.vector.tensor_copy(kT4[:, :, blk * BLOCK:(blk + 1) * BLOCK],
                                      tpk[:].rearrange("p (c b) -> p c b", c=KV))

                # ---- v block: cast straight into vb slots ----
                vf32 = natkv_pool.tile([128, KV * D], F32, tag="vf32", name=f"vf32_{blk}")
                nc.sync.dma_start(vf32[:], v_d[rows].rearrange("p c d -> p (c d)"))
                vb4 = vb[:].rearrange("p (c j x) -> p c j x", c=KV, j=NBLK)
                nc.vector.tensor_copy(vb4[:, :, blk, 0:D],
                                      vf32[:].rearrange("p (c d) -> p c d", c=KV))

        # ---- attention: 4 heads (one kv group) per pass, i-outer, j in pairs.
        # Emission is software-pipelined one chunk deep: chunk n's S^T matmuls
        # are emitted BEFORE chunk n-1's exp/PV, so TensorE always has
        # independent work while ScalarE runs exp.
        job_idx = 0
        with (
            tc.tile_pool(name="stp", bufs=2, space="PSUM") as st_pool,
            tc.tile_pool(name="oap", bufs=4, space="PSUM") as oacc_pool,
        ):
            ncols = G * BLOCK  # 512 q columns (4 heads) per S^T matmul

            # chunk list: (kv, i, jc, jn, first, last) — i-major so row i only
            # depends on blocks <= i having been loaded/transposed
            chunks = []
            for i in range(NBLK):
                for kv in range(KV):
                    jcs = list(range(0, i + 1, 2))
                    for jc in jcs:
                        chunks.append((kv, i, jc, min(2, i + 1 - jc),
                                       jc == 0, jc == jcs[-1]))

            oas_map = {}
            pending = None  # (chunk, st, pt)

            def emit_stage2(chunk, st, pt):
                """exp + PV (+ normalize/store on last chunk of an i-row)."""
                nonlocal job_idx
                kv, i, jc, jn, first, last = chunk
                h0 = kv * G
                nc.scalar.activation(pt[:, :jn * ncols], st[:, :jn * ncols],
                                     mybir.ActivationFunctionType.Exp, scale=SCALE)
                oas = oas_map[(kv, i)]
                for jj in range(jn):
                    j = jc + jj
                    vslot = (kv * NBLK + j) * VST
                    for h4 in range(G):
                        nc.tensor.matmul(
                            oas[h4][:],
                            lhsT=pt[:, jj * ncols + h4 * BLOCK: jj * ncols + (h4 + 1) * BLOCK],
                            rhs=vb[:, vslot: vslot + D + 1],
                            start=(j == 0), stop=(j == i),
                        )
                if last:
                    osb = osb_pool.tile([128, G * D], F32, tag="osb", name=f"osb_{kv}_{i}")
                    for h4 in range(G):
                        oa = oas[h4]
                        rcp = small_pool.tile([128, 1], F32, tag="rcp", name=f"rcp_{kv}_{i}_{h4}")
                        nc.vector.reciprocal(rcp[:], oa[:, D: D + 1])
                        nc.vector.tensor_scalar_mul(osb[:, h4 * D:(h4 + 1) * D], oa[:, 0: D], rcp[:])
                    nc.sync.dma_start(
                        o_d[i * BLOCK:(i + 1) * BLOCK, h0 * D:(h0 + G) * D], osb[:])
                    del oas_map[(kv, i)]
                    if job_idx < len(cache_jobs) and i % 2 == 1:
                        dst, src = cache_jobs[job_idx]
                        nc.sync.dma_start(dst, src)
                        job_idx += 1

            for chunk in chunks:
                kv, i, jc, jn, first, last = chunk
                h0 = kv * G
                if first:
                    oas_map[(kv, i)] = [
                        oacc_pool.tile([128, D + 1], F32, tag="oacc",
                                       name=f"oacc_kv{kv}_i{i}_h{h4}") for h4 in range(G)]
                st = st_pool.tile([128, 1024], F32, tag="st", name=f"st_{kv}_{i}_{jc}")
                pt = pt_pool.tile([128, 1024], F16, tag="pt", name=f"pt_{kv}_{i}_{jc}")
                for jj in range(jn):
                    j = jc + jj
                    nc.tensor.matmul(
                        st[:, jj * ncols: (jj + 1) * ncols],
                        lhsT=kT[:, kv * S + j * BLOCK: kv * S + (j + 1) * BLOCK],
                        rhs=qT[:, (i * H + h0) * BLOCK: (i * H + h0 + G) * BLOCK],
                        start=True, stop=True,
                    )
                if pending is not None:
                    emit_stage2(*pending)
                pending = (chunk, st, pt)
            if pending is not None:
                emit_stage2(*pending)

            # any leftover cache jobs
            while job_idx < len(cache_jobs):
                dst, src = cache_jobs[job_idx]
                nc.sync.dma_start(dst, src)
                job_idx += 1


_PROG = None


def build_program():
    global _PROG
    if _PROG is None:
        nc = bacc.Bacc("TRN2", target_bir_lowering=False, debug=False,
                       num_devices=N_CORES)
        with tile.TileContext(nc) as tc:
            _emit(nc, tc)
        nc.compile()
        _PROG = nc
    return _PROG


def make_in_maps(q, k, v, k_cache, v_cache):
    in_maps = []
    for c in range(N_CORES):
        sl = slice(c * S, (c + 1) * S)
        un = slice(T + c * UN_PER_CORE, T + (c + 1) * UN_PER_CORE)
        in_maps.append({
            "q": np.ascontiguousarray(q[sl]),
            "k": np.ascontiguousarray(k[sl]),
            "v": np.ascontiguousarray(v[sl]),
            "kc_un": np.ascontiguousarray(k_cache[un]),
            "vc_un": np.ascontiguousarray(v_cache[un]),
        })
    return in_maps


def _gather(results, k_cache, v_cache):
    o = np.concatenate([results[c]["o"] for c in range(N_CORES)], axis=0)
    kc = np.empty((NUM_SLOTS, KV, D), np.float32)
    vc = np.empty((NUM_SLOTS, KV, D), np.float32)
    for c in range(N_CORES):
        kc[c * S:(c + 1) * S] = results[c]["kc_out"][:S]
        vc[c * S:(c + 1) * S] = results[c]["vc_out"][:S]
        kc[T + c * UN_PER_CORE: T + (c + 1) * UN_PER_CORE] = results[c]["kc_out"][S:]
        vc[T + c * UN_PER_CORE: T + (c + 1) * UN_PER_CORE] = results[c]["vc_out"][S:]
    return o, kc, vc


def _numpy_fallback(q, k, v, k_cache, v_cache, slot_mapping, cu_seqlens_q):
    """Exact reference semantics in numpy, used only if inputs deviate from
    the hardcoded fast-path layout."""
    kc = k_cache.copy()
    vc = v_cache.copy()
    kc[slot_mapping] = k
    vc[slot_mapping] = v
    b = cu_seqlens_q.shape[0] - 1
    s = q.shape[0] // b
    qb = q.reshape(b, s, H, D)
    kb = np.repeat(k.reshape(b, s, KV, D), G, axis=2)
    vb_ = np.repeat(v.reshape(b, s, KV, D), G, axis=2)
    blk = np.arange(s) // BLOCK
    mask = blk[:, None] >= blk[None, :]
    scores = np.einsum("bqhd,bkhd->bhqk", qb * SCALE, kb)
    scores = np.where(mask[None, None], scores, np.finfo(np.float32).min)
    scores = scores - scores.max(-1, keepdims=True)
    p = np.exp(scores)
    p = p / p.sum(-1, keepdims=True)
    o = np.einsum("bhqk,bkhd->bqhd", p, vb_).astype(np.float32)
    return o.reshape(b * s, H * D), kc, vc


def kernel(q, k, v, k_cache, v_cache, slot_mapping, cu_seqlens_q, cu_seqlens_k,
           block_size):
    q = np.asarray(q, np.float32)
    k = np.asarray(k, np.float32)
    v = np.asarray(v, np.float32)
    k_cache = np.asarray(k_cache, np.float32)
    v_cache = np.asarray(v_cache, np.float32)
    slot_mapping = np.asarray(slot_mapping)
    cu_seqlens_q = np.asarray(cu_seqlens_q)
    cu_seqlens_k = np.asarray(cu_seqlens_k)

    fast = (
        q.shape == (T, H, D) and k.shape == (T, KV, D) and v.shape == (T, KV, D)
        and k_cache.shape == (NUM_SLOTS, KV, D) and v_cache.shape == (NUM_SLOTS, KV, D)
        and int(np.asarray(block_size)) == BLOCK
        and np.array_equal(slot_mapping, np.arange(T, dtype=slot_mapping.dtype))
        and np.array_equal(cu_seqlens_q, np.arange(B + 1, dtype=cu_seqlens_q.dtype) * S)
        and np.array_equal(cu_seqlens_k, np.arange(B + 1, dtype=cu_seqlens_k.dtype) * S)
    )
    if not fast:
        return _numpy_fallback(q, k, v, k_cache, v_cache, slot_mapping, cu_seqlens_q)

    nc = build_program()
    in_maps = make_in_maps(q, k, v, k_cache, v_cache)
    res = bass_utils.run_bass_kernel_spmd(nc, in_maps, core_ids=list(range(N_CORES)))
    return _gather(res.results, k_cache, v_cache)


if __name__ == "__main__":
    rng = np.random.default_rng(0)
    q = rng.standard_normal((T, H, D), dtype=np.float32)
    k = rng.standard_normal((T, KV, D), dtype=np.float32)
    v = rng.standard_normal((T, KV, D), dtype=np.float32)
    kc = np.zeros((NUM_SLOTS, KV, D), np.float32)
    vc = np.zeros((NUM_SLOTS, KV, D), np.float32)
    sm = np.arange(T, dtype=np.int32)
    cu = np.arange(B + 1, dtype=np.int32) * S
    o, kcn, vcn = kernel(q=q, k=k, v=v, k_cache=kc, v_cache=vc, slot_mapping=sm,
                         cu_seqlens_q=cu, cu_seqlens_k=cu, block_size=128)
    oref, kref, vref = _numpy_fallback(q, k, v, kc, vc, sm, cu)
    print("o relerr:", np.abs(o - oref).max() / np.abs(oref).max())
    print("kc equal:", np.array_equal(kcn, kref), "vc equal:", np.array_equal(vcn, vref))
